# revision 6
# baseline (speedup 1.0000x reference)
"""GCN encoder (3-layer GCNConv + BatchNorm + ReLU + global mean pool) on 8
Trainium2 NeuronCores.

Strategy (graph/data parallel, edges sharded by destination):
  - Nodes are split into 8 contiguous shards (one per core). Each core owns
    all edges whose destination lands in its shard.
  - The layer is computed aggregate-first (mathematically identical to the
    reference's transform-first order since GCNConv is linear). The GCN edge
    norm enorm = dis[src]*dis[dst] is FACTORED: table rows are pre-scaled by
    dis[src] (t[v] = dis[v]*h[v]), the self loop becomes an identity-matmul
    of the pre-scaled local features, and dis[dst] is applied once to the
    aggregated zT with a free-axis broadcast multiply:
        zT[:, v] = dis[v] * ( sum_{e: dst=v} t[src_e]  +  t[v] )
        hpre  = W.T @ zT                         (kept transposed: [D, nodes])
        h_out = relu(gamma * (hpre - mu) / sqrt(var+eps) + beta)
        t_out = dis * h_out                      (pre-scale for next layer)
    This makes the per-128-edge-tile selection matrix BINARY {0,1}, so it is
    stored in fp8 (exact) — HALF the bytes of f16 — and kept RESIDENT in
    SBUF: streamed from HBM once during layer 0 and reused by layers 1-2,
    removing ~22.5 MB/layer/core of HBM traffic vs streaming f16 Sel.
  - The gather t[src_e] uses dma_gather (int16 indices, 4096 rows per
    instruction, single_packet=False) from a replicated node table in HBM.
    The table is stored as two tensors in AllGather order (half A = every
    core's first HA shard rows, half B = the rest) so that (a) each half
    stays under the int16 index range and (b) the next layer's phase-A
    gathers only depend on AG(A), overlapping with AG(B) in flight.
  - Per 128-edge tile the scatter-add is a PE matmul G.T @ Sel accumulated
    in PSUM over a 128-destination window; the self-loop is an fp8 identity
    matmul against the previous layer's (pre-scaled) activations in SBUF.
  - BatchNorm statistics are free-axis reductions in the transposed layout;
    partials are combined with a [128,2] AllReduce. After normalization the
    result is scaled by dis (except the last layer, which feeds pooling
    unscaled), transposed back (PE transpose) and AllGathered into the next
    layer's node table.
  - Mean pooling reuses the selection-matmul trick against the sorted graph
    ids, followed by a [128,256] AllReduce and division by counts.
"""

import sys

sys.path.insert(0, "/opt/trn_rl_repo")

import numpy as np

import concourse.bass as bass
import concourse.tile as tile
from concourse import bacc, mybir
from concourse import bass_utils
from concourse.masks import make_identity

F32 = mybir.dt.float32
F16 = mybir.dt.float16
F8 = mybir.dt.float8e4
I16 = mybir.dt.int16
NP8 = mybir.dt.np(mybir.dt.float8e4)
OP = mybir.AluOpType
ACTF = mybir.ActivationFunctionType

NCORES = 8
D = 128
P = 128
GB_TILES = 32     # 128-edge tiles per dma_gather
WBLK = 512        # node columns per W-matmul / BN block
EPS = 1e-5


class Cfg:
    def __init__(self, N, E, G, L=3):
        assert N % NCORES == 0
        self.N, self.E, self.G, self.L = N, E, G, L
        self.NP = N // NCORES                    # nodes per core
        self.nwin = -(-self.NP // P)             # 128-dst windows per core
        assert self.nwin >= 2
        # each shard splits into half A (first NFA full node tiles) and
        # half B; the two AllGathers pipeline against the next layer's
        # phase-A gathers
        self.NFA = self.nwin // 2
        self.HA = self.NFA * P
        self.HB = self.NP - self.HA
        assert NCORES * max(self.HA, self.HB) < 32768
        self.winlens = [min(P, self.NP - w * P) for w in range(self.nwin)]
        self.nblk = -(-self.NP // WBLK)          # 512-node BN/W blocks
        self.blens = [min(WBLK, self.NP - b * WBLK) for b in range(self.nblk)]
        self.nfull = self.NP // P                # full 128-node tiles
        self.rem = self.NP - self.nfull * P
        self.gblk = -(-G // P)                   # 128-graph output tiles
        assert self.gblk * P == G or G <= P


def host_preprocess(cfg, x, edge_index, batch, Ws, bs, gammas, betas):
    """Shard + sort edges, build per-core packed metadata arrays."""
    N, G = cfg.N, cfg.G
    NP = cfg.NP
    x = np.ascontiguousarray(np.asarray(x, np.float32))
    src = np.asarray(edge_index[0]).astype(np.int64)
    dst = np.asarray(edge_index[1]).astype(np.int64)
    batch = np.asarray(batch).astype(np.int64)

    deg = (1.0 + np.bincount(dst, minlength=N)).astype(np.float32)
    dis = (1.0 / np.sqrt(deg)).astype(np.float32)

    counts = np.bincount(batch, minlength=G).astype(np.float32)
    recip = (1.0 / np.maximum(counts, 1.0)).astype(np.float32)

    # node features pre-scaled by dis (the gather-table rows / self operand)
    xsc = x * dis[:, None]

    # per-core edge lists sharded by dst, sorted by (half, local dst);
    # the gather table is stored in AllGather order: half A = concat of all
    # cores' first HA rows, half B = concat of the rest
    per_core = []
    core_of = dst // NP
    for c in range(NCORES):
        m = core_of == c
        s, dl = src[m], dst[m] - c * NP
        sc = s // NP
        sl = s - sc * NP
        h = (sl >= cfg.HA).astype(np.int64)
        rel = np.where(h == 0, sc * cfg.HA + sl, sc * cfg.HB + (sl - cfg.HA))
        order = np.lexsort((dl, h))
        per_core.append((rel[order], dl[order], h[order]))

    # shared static tile schedule: per (window, half), max tiles over cores
    nwin = cfg.nwin
    cnt = np.zeros((NCORES, nwin, 2), np.int64)
    bounds = []
    for c in range(NCORES):
        s, dl, h = per_core[c]
        nlo = int(np.searchsorted(h, 1))
        blo = np.searchsorted(dl[:nlo], np.arange(nwin + 1) * P)
        bhi = nlo + np.searchsorted(dl[nlo:], np.arange(nwin + 1) * P)
        bounds.append((blo, bhi))
        cnt[c, :, 0] = blo[1:] - blo[:-1]
        cnt[c, :, 1] = bhi[1:] - bhi[:-1]
    tiles_wh = np.max(-(-cnt // P), axis=0)      # [nwin, 2]
    T_lo = int(tiles_wh[:, 0].sum())
    T_hi = int(tiles_wh[:, 1].sum())
    T = T_lo + T_hi

    src_rel = np.zeros((NCORES, P, T), np.int16)
    # dense layer-invariant BINARY selection tiles in fp8 (exact 0/1),
    # streamed from HBM once in layer 0 and SBUF-resident afterwards:
    # selbig[:, t*P:(t+1)*P][r, c] = 1 for tile t's edge in slot r targeting
    # window-local dst c (zeros elsewhere)
    selbig = np.zeros((NCORES, P, T * P), NP8)
    for c in range(NCORES):
        s, dl, h = per_core[c]
        blo, bhi = bounds[c]
        for half in (0, 1):
            t0 = 0 if half == 0 else T_lo
            bb = blo if half == 0 else bhi
            for wi in range(nwin):
                for j in range(int(tiles_wh[wi, half])):
                    a = bb[wi] + j * P
                    n = max(0, min(P, bb[wi + 1] - a))
                    t = t0 + j
                    if n > 0:
                        src_rel[c, :n, t] = s[a:a + n]
                        selbig[c, np.arange(n),
                               t * P + (dl[a:a + n] - wi * P)] = 1.0
                t0 += int(tiles_wh[wi, half])

    # int16 index stream for dma_gather: flat position i -> [i%16, i//16],
    # replicated across the 8 16-partition groups
    idx16 = np.zeros((NCORES, P, 8 * T), np.int16)
    for c in range(NCORES):
        flat = src_rel[c].T.reshape(-1)          # tile-major, then partition
        wrapped = flat.reshape(-1, 16).T         # [16, 8*T]
        idx16[c] = np.tile(wrapped, (8, 1))

    # per-core own-shard pre-scaled features in [node%128, tile*128+d]
    # layout (layer-0 self-loop operand, loaded straight into hnew),
    # zero-padded tail
    NT = nwin
    x_own = np.zeros((NCORES, P, NT * P), np.float16)
    batchf = np.full((NCORES, P, NT), -1.0, np.float32)
    disb = np.zeros((NCORES, P, NT * P), np.float16)
    for c in range(NCORES):
        xs = xsc[c * NP:(c + 1) * NP]
        pad = np.zeros((NT * P - NP, D), np.float32)
        x_own[c] = np.concatenate([xs, pad]).reshape(NT, P, D).transpose(
            1, 0, 2).reshape(P, NT * P).astype(np.float16)
        ids = np.concatenate([batch[c * NP:(c + 1) * NP].astype(np.float32),
                              np.full(NT * P - NP, -1.0, np.float32)])
        batchf[c] = ids.reshape(NT, P).T
        row = np.concatenate([dis[c * NP:(c + 1) * NP],
                              np.zeros(NT * P - NP, np.float32)])
        disb[c] = np.broadcast_to(row.astype(np.float16), (P, NT * P))

    iota = np.broadcast_to(np.arange(512, dtype=np.float16), (P, 512)).copy()
    identf8 = np.eye(P, dtype=np.float32).astype(NP8)
    Wpack = np.asarray(Ws, np.float16).transpose(1, 0, 2).reshape(D, cfg.L * D)
    gb = np.zeros((P, 2 * cfg.L), np.float32)
    for l in range(cfg.L):
        gb[:, 2 * l] = np.asarray(gammas[l], np.float32)
        gb[:, 2 * l + 1] = np.asarray(betas[l], np.float32)
    recip_pk = np.zeros((P, cfg.gblk), np.float32)
    for b in range(cfg.gblk):
        n = min(P, G - b * P)
        recip_pk[:n, b] = recip[b * P:b * P + n]

    xa = np.concatenate([xsc[c * NP:c * NP + cfg.HA]
                         for c in range(NCORES)]).astype(np.float16)
    xb = np.concatenate([xsc[c * NP + cfg.HA:(c + 1) * NP]
                         for c in range(NCORES)]).astype(np.float16)
    shared = dict(xa=np.ascontiguousarray(xa), xb=np.ascontiguousarray(xb),
                  wt=Wpack, gb=gb, iota=iota, recip=recip_pk,
                  identf8=identf8)
    per_core_inputs = [dict(idx16=np.ascontiguousarray(idx16[c]),
                            selbig=np.ascontiguousarray(selbig[c]),
                            batchf=np.ascontiguousarray(batchf[c]),
                            xown=np.ascontiguousarray(x_own[c]),
                            disb=np.ascontiguousarray(disb[c]))
                       for c in range(NCORES)]
    sched = dict(tiles_wh=tiles_wh, T_lo=T_lo, T_hi=T_hi, T=T)
    return shared, per_core_inputs, sched


def build(cfg, sched, debug_dump=False):
    tiles_wh = sched["tiles_wh"]
    T_lo, T_hi, T = sched["T_lo"], sched["T_hi"], sched["T"]
    L, N, G, NP = cfg.L, cfg.N, cfg.G, cfg.NP

    nc = bacc.Bacc("TRN2", target_bir_lowering=False, debug=False,
                   num_devices=NCORES, num_swdge_queues=4,
                   dynamic_dma_scratch_size=16384)
    dbg = {}
    if debug_dump:
        for nm, shape, dt_ in [("zT", [P, cfg.nwin * P], F16),
                               ("hpre", [P, cfg.nblk * WBLK], F16),
                               ("stat", [P, 2], F32),
                               ("hnew", [P, cfg.nwin * P], F16)]:
            for l in range(L):
                dbg[f"{nm}{l}"] = nc.dram_tensor(
                    f"dbg_{nm}{l}", shape, dt_, kind="ExternalOutput")

    xa_e = nc.dram_tensor("xa", [NCORES * cfg.HA, D], F16,
                          kind="ExternalInput")
    xb_e = nc.dram_tensor("xb", [NCORES * cfg.HB, D], F16,
                          kind="ExternalInput")
    wt_e = nc.dram_tensor("wt", [P, L * D], F16, kind="ExternalInput")
    gb_e = nc.dram_tensor("gb", [P, 2 * L], F32, kind="ExternalInput")
    iota_e = nc.dram_tensor("iota", [P, 512], F16, kind="ExternalInput")
    recip_e = nc.dram_tensor("recip", [P, cfg.gblk], F32, kind="ExternalInput")
    identf8_e = nc.dram_tensor("identf8", [P, P], F8, kind="ExternalInput")
    batchf_e = nc.dram_tensor("batchf", [P, cfg.nwin], F32, kind="ExternalInput")
    idx16_e = nc.dram_tensor("idx16", [P, 8 * T], I16, kind="ExternalInput")
    selbig_e = nc.dram_tensor("selbig", [P, T * P], F8, kind="ExternalInput")
    xown_e = nc.dram_tensor("xown", [P, cfg.nwin * P], F16, kind="ExternalInput")
    disb_e = nc.dram_tensor("disb", [P, cfg.nwin * P], F16, kind="ExternalInput")
    out_e = nc.dram_tensor("out", [G, D], F32, kind="ExternalOutput")

    rg = [list(range(NCORES))]

    with tile.TileContext(nc) as tc:
        with tc.tile_pool(name="const", bufs=1) as cp, \
             tc.tile_pool(name="gpool", bufs=6) as gp, \
             tc.tile_pool(name="mselp", bufs=3) as mselp, \
             tc.tile_pool(name="big", bufs=1) as bigp, \
             tc.tile_pool(name="scr", bufs=2) as scrp, \
             tc.tile_pool(name="small", bufs=4) as smp, \
             tc.tile_pool(name="pz", bufs=4, space="PSUM") as pzp, \
             tc.tile_pool(name="ph", bufs=1, space="PSUM") as php, \
             tc.tile_pool(name="pt", bufs=2, space="PSUM") as ptp, \
             tc.tile_pool(name="pg", bufs=1, space="PSUM") as pgp, \
             tc.tile_pool(name="dram", bufs=1, space="DRAM") as dp:

            # ---- constants into SBUF ----
            iota_sb = cp.tile([P, 512], F16)
            nc.sync.dma_start(out=iota_sb[:], in_=iota_e[:, :])
            wt_sb = cp.tile([P, L * D], F16)
            nc.sync.dma_start(out=wt_sb[:], in_=wt_e[:, :])
            gb_sb = cp.tile([P, 2 * L], F32)
            nc.sync.dma_start(out=gb_sb[:], in_=gb_e[:, :])
            recip_sb = cp.tile([P, cfg.gblk], F32)
            nc.sync.dma_start(out=recip_sb[:], in_=recip_e[:, :])
            batchf_sb = cp.tile([P, cfg.nwin], F32)
            nc.sync.dma_start(out=batchf_sb[:], in_=batchf_e[:, :])
            idx_sb = cp.tile([P, 8 * T], I16)
            nc.sync.dma_start(out=idx_sb[:], in_=idx16_e[:, :])
            disb_sb = cp.tile([P, cfg.nwin * P], F16)
            nc.sync.dma_start(out=disb_sb[:], in_=disb_e[:, :])
            identf8_sb = cp.tile([P, P], F8)
            nc.sync.dma_start(out=identf8_sb[:], in_=identf8_e[:, :])
            ident = cp.tile([P, P], F32)
            make_identity(nc, ident[:])
            ident16 = cp.tile([P, P], F16)
            make_identity(nc, ident16[:])
            zero_c = cp.tile([P, 1], F32)
            nc.vector.memset(zero_c[:], 0.0)
            eps_c = cp.tile([P, 1], F32)
            nc.vector.memset(eps_c[:], EPS)

            # layer-invariant binary Sel, SBUF-resident (fp8). Streamed
            # chunk-by-chunk during layer 0, reused by layers 1-2.
            selres = cp.tile([P, T * P], F8)
            sel_loaded = {}

            def ensure_selchunk(gt, l):
                """Stream the prebuilt Sel chunk covering GLOBAL tile gt
                into the resident buffer (layer 0 only)."""
                if l > 0:
                    return
                k = gt // GB_TILES
                if k not in sel_loaded:
                    cnt_t = min(GB_TILES, T - k * GB_TILES)
                    a = k * GB_TILES * P
                    nc.sync.dma_start(
                        out=selres[:, a:a + cnt_t * P],
                        in_=selbig_e[:, a:a + cnt_t * P])
                    sel_loaded[k] = True

            # layer-0 self operand loaded straight into hnew (pre-scaled x)
            hnew = bigp.tile([P, cfg.nwin * P], F16)
            nc.sync.dma_start(out=hnew[:], in_=xown_e[:, :])

            tabA, tabB = [], []
            for l in range(L - 1):
                tabA.append(dp.tile([NCORES * cfg.HA, D], F16,
                                    addr_space="Shared", name=f"tabA{l}"))
                tabB.append(dp.tile([NCORES * cfg.HB, D], F16,
                                    addr_space="Shared", name=f"tabB{l}"))

            # persistent num_idxs registers: one per distinct gather row
            # count, created once so gathers carry no per-instruction MOVE
            # (a fresh MOVE onto the shared register stalls dispatch until
            # the in-flight gather reading it retires)
            nidx_regs = {}
            for T_ph in (T_lo, T_hi):
                k = 0
                while k * GB_TILES < T_ph:
                    v = min(GB_TILES, T_ph - k * GB_TILES) * P
                    if v not in nidx_regs:
                        nidx_regs[v] = nc.gpsimd.to_reg(v)
                    k += 1

            # per-(window,half) phase-local first tile index
            starts = np.zeros((cfg.nwin, 2), np.int64)
            t0 = 0
            for w in range(cfg.nwin):
                starts[w, 0] = t0
                t0 += int(tiles_wh[w][0])
            t0 = 0
            for w in range(cfg.nwin):
                starts[w, 1] = t0
                t0 += int(tiles_wh[w][1])

            for l in range(L):
                srcA = xa_e if l == 0 else tabA[l - 1]
                srcB = xb_e if l == 0 else tabB[l - 1]

                zT = bigp.tile([P, cfg.nwin * P], F16, tag="zT")
                gather_tiles = [{}, {}]
                gq = [0]  # round-robin SWDGE queue over gathers

                def issue_phase_gathers(half, l=l, srcA=srcA, srcB=srcB,
                                        gather_tiles=gather_tiles):
                    """Issue ALL of a phase's dma_gathers back-to-back into
                    fresh pool slots. With no interleaved consumers or slot
                    reuse, the Tile framework attaches each gather's waits to
                    the instruction itself (no standalone EventSemaphore
                    barriers on gpsimd), letting gathers on different SWDGE
                    queues overlap descriptor generation across Q7 core
                    pairs."""
                    T_ph = T_lo if half == 0 else T_hi
                    cache = gather_tiles[half]
                    nk = -(-T_ph // GB_TILES)
                    for k in range(nk):
                        cnt_t = min(GB_TILES, T_ph - k * GB_TILES)
                        g = gp.tile([P, GB_TILES * P], F16, tag="gath")
                        cache[k] = g
                        base = (0 if half == 0 else T_lo) + k * GB_TILES
                        tbl = srcA[:, :] if half == 0 else srcB[:, :]
                        # tiny Act-engine touch: absorbs the slot's
                        # WAR/WAW waits on the near-idle scalar queue, so the
                        # gather carries a single attached wait and the
                        # gpsimd stream is pure gathers (max Q7 in-flight
                        # depth, no standalone EventSemaphore barriers)
                        nc.scalar.activation(out=g[0:1, :1],
                                             in_=zero_c[0:1, :1],
                                             func=ACTF.Copy)
                        nc.gpsimd.dma_gather(
                            out_ap=g[:, :cnt_t * P].rearrange(
                                "p (t d) -> p t d", d=P),
                            in_ap=tbl,
                            idxs_ap=idx_sb[:, base * 8:(base + cnt_t) * 8],
                            num_idxs=cnt_t * P,
                            num_idxs_reg=nidx_regs[cnt_t * P],
                            elem_size=D,
                            single_packet=False,
                            queue_num=gq[0] % 4,
                        )
                        gq[0] += 1

                def ensure_gather(half, t_local, gather_tiles=gather_tiles):
                    return gather_tiles[half][t_local // GB_TILES]

                issue_phase_gathers(0)
                issue_phase_gathers(1)

                # --- lo phase: identity self-loop + lo-half edge tiles ---
                for w in range(cfg.nwin):
                    wlen = cfg.winlens[w]
                    nlo = int(tiles_wh[w][0])
                    pz = pzp.tile([P, P], F32, tag="pz")
                    nc.tensor.matmul(out=pz[:, :wlen],
                                     lhsT=hnew[:, w * P:(w + 1) * P],
                                     rhs=identf8_sb[:, :wlen],
                                     start=True, stop=(nlo == 0))
                    for j in range(nlo):
                        t = int(starts[w, 0]) + j
                        g = ensure_gather(0, t)
                        gt = t                      # global tile index
                        ensure_selchunk(gt, l)
                        slot = t % GB_TILES
                        nc.tensor.matmul(
                            out=pz[:, :wlen],
                            lhsT=g[:, slot * P:(slot + 1) * P],
                            rhs=selres[:, gt * P:gt * P + wlen],
                            start=False, stop=(j == nlo - 1))
                    # evacuate on the near-idle scalar engine: keeps the
                    # PSUM-free chain (which paces gather slot recycling)
                    # off the busier DVE queue
                    nc.scalar.activation(out=zT[:, w * P:w * P + wlen],
                                         in_=pz[:, :wlen], func=ACTF.Copy)

                # --- hi phase: hi-half edge tiles, added into zT ---
                for w in range(cfg.nwin):
                    wlen = cfg.winlens[w]
                    nhi = int(tiles_wh[w][1])
                    if nhi == 0:
                        continue
                    pz = pzp.tile([P, P], F32, tag="pz")
                    for j in range(nhi):
                        t = int(starts[w, 1]) + j
                        g = ensure_gather(1, t)
                        gt = T_lo + t               # global tile index
                        ensure_selchunk(gt, l)
                        slot = t % GB_TILES
                        nc.tensor.matmul(
                            out=pz[:, :wlen],
                            lhsT=g[:, slot * P:(slot + 1) * P],
                            rhs=selres[:, gt * P:gt * P + wlen],
                            start=(j == 0), stop=(j == nhi - 1))
                    nc.vector.tensor_tensor(out=zT[:, w * P:w * P + wlen],
                                            in0=zT[:, w * P:w * P + wlen],
                                            in1=pz[:, :wlen], op=OP.add)

                # ---- dis[dst] scale of the aggregate ----
                nc.vector.tensor_tensor(out=zT[:, :NP], in0=zT[:, :NP],
                                        in1=disb_sb[:, :NP], op=OP.mult)

                if debug_dump:
                    nc.sync.dma_start(out=dbg[f"zT{l}"][:, :], in_=zT[:])

                # ---- W matmul + BN stats ----
                hpre = bigp.tile([P, cfg.nblk * WBLK], F16, tag="hpre")
                sums = smp.tile([P, cfg.nblk], F32, tag="sums")
                sqs = smp.tile([P, cfg.nblk], F32, tag="sqs")
                for b in range(cfg.nblk):
                    blen = cfg.blens[b]
                    ph = php.tile([P, WBLK], F32, tag="ph")
                    nc.tensor.matmul(
                        out=ph[:, :blen],
                        lhsT=wt_sb[:, l * D:(l + 1) * D],
                        rhs=zT[:, b * WBLK:b * WBLK + blen],
                        start=True, stop=True)
                    nc.scalar.activation(
                        out=hpre[:, b * WBLK:b * WBLK + blen],
                        in_=ph[:, :blen], func=ACTF.Copy,
                        accum_out=sums[:, b:b + 1])
                    scr = scrp.tile([P, WBLK], F32, tag="scr")
                    nc.scalar.activation(
                        out=scr[:, :blen], in_=ph[:, :blen], func=ACTF.Square,
                        bias=zero_c[:, :1], accum_out=sqs[:, b:b + 1])

                ssum = smp.tile([P, 1], F32, tag="ssum")
                ssq = smp.tile([P, 1], F32, tag="ssq")
                nc.vector.reduce_sum(out=ssum[:], in_=sums[:],
                                     axis=mybir.AxisListType.X)
                nc.vector.reduce_sum(out=ssq[:], in_=sqs[:],
                                     axis=mybir.AxisListType.X)
                statpk = smp.tile([P, 2], F32, tag="statpk")
                nc.vector.tensor_copy(out=statpk[:, 0:1], in_=ssum[:])
                nc.vector.tensor_copy(out=statpk[:, 1:2], in_=ssq[:])
                stat_in = dp.tile([P, 2], F32, name=f"statin{l}")
                stat_out = dp.tile([P, 2], F32, addr_space="Shared",
                                   name=f"statout{l}")
                nc.sync.dma_start(out=stat_in[:], in_=statpk[:])
                nc.gpsimd.collective_compute(
                    "AllReduce", OP.add, replica_groups=rg,
                    ins=[stat_in[:].opt()], outs=[stat_out[:].opt()])
                statred = smp.tile([P, 2], F32, tag="statred")
                nc.sync.dma_start(out=statred[:], in_=stat_out[:])

                if debug_dump:
                    nc.sync.dma_start(out=dbg[f"hpre{l}"][:, :], in_=hpre[:])
                    nc.sync.dma_start(out=dbg[f"stat{l}"][:, :], in_=statred[:])

                mu = smp.tile([P, 1], F32, tag="mu")
                ex2 = smp.tile([P, 1], F32, tag="ex2")
                var = smp.tile([P, 1], F32, tag="var")
                std = smp.tile([P, 1], F32, tag="std")
                rsinv = smp.tile([P, 1], F32, tag="rsinv")
                s1 = smp.tile([P, 1], F32, tag="s1")
                s2 = smp.tile([P, 1], F32, tag="s2")
                inv_n = float(np.float32(1.0 / N))
                nc.vector.tensor_scalar(out=mu[:], in0=statred[:, 0:1],
                                        scalar1=inv_n, scalar2=None,
                                        op0=OP.mult)
                nc.vector.tensor_scalar(out=ex2[:], in0=statred[:, 1:2],
                                        scalar1=inv_n, scalar2=None,
                                        op0=OP.mult)
                nc.vector.scalar_tensor_tensor(
                    out=var[:], in0=mu[:], scalar=1.0, in1=mu[:],
                    op0=OP.bypass, op1=OP.mult)
                nc.vector.tensor_tensor(out=var[:], in0=ex2[:], in1=var[:],
                                        op=OP.subtract)
                nc.scalar.activation(out=std[:], in_=var[:], func=ACTF.Sqrt,
                                     bias=eps_c[:, :1])
                nc.vector.reciprocal(out=rsinv[:], in_=std[:])
                nc.vector.tensor_tensor(out=s1[:], in0=gb_sb[:, 2 * l:2 * l + 1],
                                        in1=rsinv[:], op=OP.mult)
                nc.vector.tensor_tensor(out=s2[:], in0=mu[:], in1=s1[:],
                                        op=OP.mult)
                nc.vector.tensor_tensor(out=s2[:],
                                        in0=gb_sb[:, 2 * l + 1:2 * l + 2],
                                        in1=s2[:], op=OP.subtract)

                # ---- normalize (+relu; + dis pre-scale except last layer),
                # transpose back to [node, D] ----
                hnorm = bigp.tile([P, cfg.nwin * P], F16, tag="zT")
                for b in range(cfg.nblk):
                    blen = cfg.blens[b]
                    sl = slice(b * WBLK, b * WBLK + blen)
                    if l < L - 1:
                        nc.scalar.activation(out=hnorm[:, sl], in_=hpre[:, sl],
                                             func=ACTF.Relu, bias=s2[:, :1],
                                             scale=s1[:, :1])
                        nc.vector.tensor_tensor(out=hnorm[:, sl],
                                                in0=hnorm[:, sl],
                                                in1=disb_sb[:, sl],
                                                op=OP.mult)
                    else:
                        nc.vector.tensor_scalar(out=hnorm[:, sl],
                                                in0=hpre[:, sl],
                                                scalar1=s1[:, :1],
                                                scalar2=s2[:, :1],
                                                op0=OP.mult, op1=OP.add)
                for nt in range(cfg.nwin):
                    tl = cfg.winlens[nt]
                    pt = ptp.tile([P, P], F16, tag="pt")
                    nc.tensor.transpose(out=pt[:tl, :],
                                        in_=hnorm[:, nt * P:nt * P + tl],
                                        identity=ident16[:])
                    nc.vector.tensor_copy(out=hnew[:tl, nt * P:(nt + 1) * P],
                                          in_=pt[:tl, :])
                    if l < L - 1 and nt == cfg.NFA - 1:
                        # half A written: AllGather it now so the next
                        # layer's phase-A gathers overlap with AG of half B
                        partA = dp.tile([cfg.HA, D], F16, name=f"partA{l}")
                        nc.sync.dma_start(
                            out=partA[:, :].rearrange("(nt p) d -> p nt d",
                                                      p=P),
                            in_=hnew[:, :cfg.NFA * P].rearrange(
                                "p (nt d) -> p nt d", d=D))
                        nc.gpsimd.collective_compute(
                            "AllGather", OP.bypass, replica_groups=rg,
                            ins=[partA[:].opt()], outs=[tabA[l][:].opt()])
                if debug_dump:
                    nc.sync.dma_start(out=dbg[f"hnew{l}"][:, :], in_=hnew[:])

                if l < L - 1:
                    partB = dp.tile([cfg.HB, D], F16, name=f"partB{l}")
                    nfb = cfg.nfull - cfg.NFA
                    if nfb:
                        nc.sync.dma_start(
                            out=partB[:nfb * P, :].rearrange(
                                "(nt p) d -> p nt d", p=P),
                            in_=hnew[:, cfg.NFA * P:cfg.nfull * P].rearrange(
                                "p (nt d) -> p nt d", d=D))
                    if cfg.rem:
                        nc.sync.dma_start(
                            out=partB[nfb * P:, :],
                            in_=hnew[:cfg.rem,
                                     cfg.nfull * P:(cfg.nfull + 1) * P])
                    nc.gpsimd.collective_compute(
                        "AllGather", OP.bypass, replica_groups=rg,
                        ins=[partB[:].opt()], outs=[tabB[l][:].opt()])

            # ---- global mean pool ----
            pgps = pgp.tile([P, G], F32)
            for nt in range(cfg.nwin):
                msel = mselp.tile([P, G], F16, tag="msel")
                nc.vector.tensor_scalar(out=msel[:], in0=iota_sb[:, :G],
                                        scalar1=batchf_sb[:, nt:nt + 1],
                                        scalar2=None, op0=OP.is_equal)
                nc.tensor.matmul(out=pgps[:], lhsT=hnew[:, nt * P:(nt + 1) * P],
                                 rhs=msel[:], start=(nt == 0),
                                 stop=(nt == cfg.nwin - 1))
            poolsb = cp.tile([P, G], F32)
            nc.vector.tensor_copy(out=poolsb[:], in_=pgps[:])
            pool_in = dp.tile([P, G], F32, name="poolin")
            pool_out = dp.tile([P, G], F32, addr_space="Shared", name="poolout")
            nc.sync.dma_start(out=pool_in[:], in_=poolsb[:])
            nc.gpsimd.collective_compute(
                "AllReduce", OP.add, replica_groups=rg,
                ins=[pool_in[:].opt()], outs=[pool_out[:].opt()])
            poolred = cp.tile([P, G], F32)
            nc.sync.dma_start(out=poolred[:], in_=pool_out[:])
            outsb = cp.tile([P, cfg.gblk * D], F32)
            for b in range(cfg.gblk):
                gl = min(P, G - b * P)
                pt = ptp.tile([P, P], F32, tag="pt")
                nc.tensor.transpose(out=pt[:gl, :],
                                    in_=poolred[:, b * P:b * P + gl],
                                    identity=ident[:])
                nc.vector.tensor_scalar(out=outsb[:gl, b * D:(b + 1) * D],
                                        in0=pt[:gl, :],
                                        scalar1=recip_sb[:gl, b:b + 1],
                                        scalar2=None, op0=OP.mult)
            if cfg.gblk == 1:
                nc.sync.dma_start(out=out_e[:, :], in_=outsb[:G, :D])
            else:
                nc.sync.dma_start(
                    out=out_e[:, :].rearrange("(b g) d -> g b d", g=P),
                    in_=outsb[:, :].rearrange("g (b d) -> g b d", d=D))
    nc.compile()
    return nc


_CACHE = {}


def _get_compiled(cfg, sched_key, sched, debug_dump=False):
    key = (cfg.N, cfg.E, cfg.G, cfg.L, sched_key, debug_dump)
    if key not in _CACHE:
        _CACHE[key] = build(cfg, sched, debug_dump=debug_dump)
    return _CACHE[key]


def run(cfg, inputs, trace=False, debug_dump=False):
    shared, per_core, sched = host_preprocess(cfg, **inputs)
    sched_key = (sched["T_lo"], sched["T_hi"],
                 tuple(map(tuple, sched["tiles_wh"])))
    nc = _get_compiled(cfg, sched_key, sched, debug_dump=debug_dump)
    in_maps = [dict(shared, **pc) for pc in per_core]
    res = bass_utils.run_bass_kernel_spmd(
        nc, in_maps, core_ids=list(range(NCORES)), trace=trace)
    out = res.results[0]["out"]
    return out, res


def build_null(cfg, sched):
    """Same external I/O signature as build(), trivial compute — used to
    subtract host/RPC/dispatch overhead from wall-clock timing."""
    T = sched["T"]
    L, N, G = cfg.L, cfg.N, cfg.G
    nc = bacc.Bacc("TRN2", target_bir_lowering=False, debug=False,
                   num_devices=NCORES)
    nc.dram_tensor("xa", [NCORES * cfg.HA, D], F16, kind="ExternalInput")
    nc.dram_tensor("xb", [NCORES * cfg.HB, D], F16, kind="ExternalInput")
    nc.dram_tensor("wt", [P, L * D], F16, kind="ExternalInput")
    nc.dram_tensor("gb", [P, 2 * L], F32, kind="ExternalInput")
    iota_e = nc.dram_tensor("iota", [P, 512], F16, kind="ExternalInput")
    nc.dram_tensor("recip", [P, cfg.gblk], F32, kind="ExternalInput")
    nc.dram_tensor("identf8", [P, P], F8, kind="ExternalInput")
    nc.dram_tensor("batchf", [P, cfg.nwin], F32, kind="ExternalInput")
    nc.dram_tensor("idx16", [P, 8 * T], I16, kind="ExternalInput")
    nc.dram_tensor("selbig", [P, T * P], F8, kind="ExternalInput")
    nc.dram_tensor("xown", [P, cfg.nwin * P], F16, kind="ExternalInput")
    nc.dram_tensor("disb", [P, cfg.nwin * P], F16, kind="ExternalInput")
    out_e = nc.dram_tensor("out", [G, D], F32, kind="ExternalOutput")
    with tile.TileContext(nc) as tc:
        with tc.tile_pool(name="sb", bufs=1) as sb:
            t = sb.tile([P, D], F16)
            nc.sync.dma_start(out=t[:], in_=iota_e[:, :D])
            t32 = sb.tile([P, D], F32)
            nc.vector.tensor_copy(out=t32[:], in_=t[:])
            for b in range(-(-G // P)):
                gl = min(P, G - b * P)
                nc.sync.dma_start(out=out_e[b * P:b * P + gl, :],
                                  in_=t32[:gl, :])
    nc.compile()
    return nc


def _make_pjrt_fn(nc, in_maps, chain=1, raw=False):
    """Compile the program via PJRT and return a zero-arg callable that
    executes it `chain` times back-to-back (each call's outputs feed the
    next call's output operands, forcing serial device execution) with
    device-resident inputs."""
    import jax
    from jax.sharding import Mesh, PartitionSpec, NamedSharding
    from jax.experimental.shard_map import shard_map
    from concourse import bass2jax
    from concourse import mybir as mb

    bass2jax.install_neuronx_cc_hook()
    partition_name = (nc.partition_id_tensor.name
                      if nc.partition_id_tensor else None)
    in_names, out_names, out_avals, zero_outs = [], [], [], []
    for alloc in nc.m.functions[0].allocations:
        if not isinstance(alloc, mb.MemoryLocationSet):
            continue
        name = alloc.memorylocations[0].name
        if alloc.kind == "ExternalInput":
            if name != partition_name:
                in_names.append(name)
        elif alloc.kind == "ExternalOutput":
            out_names.append(name)
            shape = tuple(alloc.tensor_shape)
            dtype = mb.dt.np(alloc.dtype)
            out_avals.append(jax.core.ShapedArray(shape, dtype))
            zero_outs.append(np.zeros(shape, dtype))
    n_params = len(in_names)
    in_names = in_names + out_names
    if partition_name is not None:
        in_names.append(partition_name)

    def _body(*args):
        params = list(args[:n_params])
        outs = list(args[n_params:])
        pid = ([bass2jax.partition_id_tensor()]
               if partition_name is not None else [])
        for _ in range(chain):
            outs = list(bass2jax._bass_exec_p.bind(
                *params, *outs, *pid,
                out_avals=tuple(out_avals), in_names=tuple(in_names),
                out_names=tuple(out_names), lowering_input_output_aliases=(),
                sim_require_finite=True, sim_require_nnan=True, nc=nc))
        return tuple(outs)

    devices = jax.devices()[:NCORES]
    mesh = Mesh(np.asarray(devices), ("core",))
    spec = PartitionSpec("core")
    in_specs = (spec,) * (n_params + len(out_names))
    out_specs = (spec,) * len(out_names)
    fn = jax.jit(shard_map(_body, mesh=mesh, in_specs=in_specs,
                           out_specs=out_specs, check_rep=False),
                 keep_unused=True)
    sharding = NamedSharding(mesh, spec)
    concat_in = [
        jax.device_put(np.concatenate(
            [np.asarray(in_maps[c][in_names[i]]) for c in range(NCORES)],
            axis=0), sharding)
        for i in range(n_params)
    ]
    concat_zeros = [
        jax.device_put(np.zeros((NCORES * z.shape[0], *z.shape[1:]), z.dtype),
                       sharding)
        for z in zero_outs
    ]
    jax.block_until_ready(concat_in)
    if raw:
        return fn, concat_in, concat_zeros

    def call():
        jax.block_until_ready(fn(*concat_in, *concat_zeros))

    return call


def time_pjrt(nc, in_maps, iters=8, warmup=2, chain=1):
    """Wall-clock repeated executions of the compiled program with
    device-resident inputs (mirrors bass2jax.run_bass_via_pjrt)."""
    import time
    call = _make_pjrt_fn(nc, in_maps, chain=chain)
    for _ in range(warmup):
        call()
    times = []
    for _ in range(iters):
        t0 = time.perf_counter()
        call()
        times.append(time.perf_counter() - t0)
    return times


def measure(cfg, inputs, iters=24):
    """Interleaved blocked kernel/null executions. The ~82ms axon RPC round
    trip dominates each call, so the kernel time is estimated as the trimmed
    mean of paired (kernel - null) wall differences; interleaving cancels
    slow drift. Returns (kernel_walls, null_walls) as paired lists."""
    import time
    import jax
    shared, per_core, sched = host_preprocess(cfg, **inputs)
    sched_key = (sched["T_lo"], sched["T_hi"],
                 tuple(map(tuple, sched["tiles_wh"])))
    nc = _get_compiled(cfg, sched_key, sched)
    in_maps = [dict(shared, **pc) for pc in per_core]
    key = ("null", cfg.N, cfg.E, cfg.G, sched_key)
    if key not in _CACHE:
        _CACHE[key] = build_null(cfg, sched)
    fnk, ink, zk = _make_pjrt_fn(nc, in_maps, raw=True)
    fnn, inn, zn = _make_pjrt_fn(_CACHE[key], in_maps, raw=True)
    jax.block_until_ready(fnk(*ink, *zk))
    jax.block_until_ready(fnn(*inn, *zn))
    ks, ns = [], []
    for _ in range(iters):
        t0 = time.perf_counter()
        jax.block_until_ready(fnk(*ink, *zk))
        ks.append(time.perf_counter() - t0)
        t0 = time.perf_counter()
        jax.block_until_ready(fnn(*inn, *zn))
        ns.append(time.perf_counter() - t0)
    return ks, ns


def kernel(x, edge_index, batch, Ws, bs, gammas, betas):
    cfg = Cfg(N=50000, E=625000, G=256, L=3)
    out, _ = run(cfg, dict(x=x, edge_index=edge_index, batch=batch, Ws=Ws,
                           bs=bs, gammas=gammas, betas=betas))
    return out


# revision 9
# speedup vs baseline: 2.1765x; 2.1765x over previous
"""GCN encoder (3-layer GCNConv + BatchNorm + ReLU + global mean pool) on 8
Trainium2 NeuronCores.

Strategy (graph/data parallel, edges sharded by destination):
  - Nodes are split into 8 contiguous shards (one per core). Each core owns
    all edges whose destination lands in its shard.
  - The layer is computed aggregate-first (mathematically identical to the
    reference's transform-first order since GCNConv is linear). The GCN edge
    norm enorm = dis[src]*dis[dst] is FACTORED: table rows are pre-scaled by
    dis[src] (t[v] = dis[v]*h[v]), the self loop becomes an identity-matmul
    of the pre-scaled local features, and dis[dst] is applied once to the
    aggregated zT with a free-axis broadcast multiply:
        zT[:, v] = dis[v] * ( sum_{e: dst=v} t[src_e]  +  t[v] )
        hpre  = W.T @ zT                         (kept transposed: [D, nodes])
        h_out = relu(gamma * (hpre - mu) / sqrt(var+eps) + beta)
        t_out = dis * h_out                      (pre-scale for next layer)
    This makes the per-128-edge-tile selection matrix BINARY {0,1}, so it is
    stored in fp8 (exact) — HALF the bytes of f16 — and kept RESIDENT in
    SBUF: streamed from HBM once during layer 0 and reused by layers 1-2,
    removing ~22.5 MB/layer/core of HBM traffic vs streaming f16 Sel.
  - The gather t[src_e] uses dma_gather (int16 indices, 4096 rows per
    instruction, single_packet=False) from a replicated node table in HBM.
    The table is stored as two tensors in AllGather order (half A = every
    core's first HA shard rows, half B = the rest) so that (a) each half
    stays under the int16 index range and (b) the next layer's phase-A
    gathers only depend on AG(A), overlapping with AG(B) in flight.
  - Per 128-edge tile the scatter-add is a PE matmul G.T @ Sel accumulated
    in PSUM over a 128-destination window; the self-loop is an fp8 identity
    matmul against the previous layer's (pre-scaled) activations in SBUF.
  - BatchNorm statistics are free-axis reductions in the transposed layout;
    partials are combined with a [128,2] AllReduce. After normalization the
    result is scaled by dis (except the last layer, which feeds pooling
    unscaled), transposed back (PE transpose) and AllGathered into the next
    layer's node table.
  - Mean pooling reuses the selection-matmul trick against the sorted graph
    ids, followed by a [128,256] AllReduce and division by counts.
"""

import sys

sys.path.insert(0, "/opt/trn_rl_repo")

import numpy as np

import concourse.bass as bass
import concourse.tile as tile
from concourse import bacc, mybir
from concourse import bass_utils
from concourse.masks import make_identity

F32 = mybir.dt.float32
F16 = mybir.dt.float16
F8 = mybir.dt.float8e4
I16 = mybir.dt.int16
NP8 = mybir.dt.np(mybir.dt.float8e4)
OP = mybir.AluOpType
ACTF = mybir.ActivationFunctionType

NCORES = 8
D = 128
P = 128
GB_TILES = 32     # 128-edge tiles per dma_gather
WBLK = 512        # node columns per W-matmul / BN block
EPS = 1e-5


class Cfg:
    def __init__(self, N, E, G, L=3):
        assert N % NCORES == 0
        self.N, self.E, self.G, self.L = N, E, G, L
        self.NP = N // NCORES                    # nodes per core
        self.nwin = -(-self.NP // P)             # 128-dst windows per core
        assert self.nwin >= 2
        # each shard splits into half A (first NFA full node tiles) and
        # half B; the two AllGathers pipeline against the next layer's
        # phase-A gathers
        self.NFA = self.nwin // 2
        self.HA = self.NFA * P
        self.HB = self.NP - self.HA
        assert NCORES * max(self.HA, self.HB) < 32768
        self.winlens = [min(P, self.NP - w * P) for w in range(self.nwin)]
        self.nblk = -(-self.NP // WBLK)          # 512-node BN/W blocks
        self.blens = [min(WBLK, self.NP - b * WBLK) for b in range(self.nblk)]
        self.nfull = self.NP // P                # full 128-node tiles
        self.rem = self.NP - self.nfull * P
        self.gblk = -(-G // P)                   # 128-graph output tiles
        assert self.gblk * P == G or G <= P


def host_preprocess(cfg, x, edge_index, batch, Ws, bs, gammas, betas):
    """Shard + sort edges, build per-core packed metadata arrays."""
    N, G = cfg.N, cfg.G
    NP = cfg.NP
    x = np.ascontiguousarray(np.asarray(x, np.float32))
    src = np.asarray(edge_index[0]).astype(np.int64)
    dst = np.asarray(edge_index[1]).astype(np.int64)
    batch = np.asarray(batch).astype(np.int64)

    deg = (1.0 + np.bincount(dst, minlength=N)).astype(np.float32)
    dis = (1.0 / np.sqrt(deg)).astype(np.float32)

    counts = np.bincount(batch, minlength=G).astype(np.float32)
    recip = (1.0 / np.maximum(counts, 1.0)).astype(np.float32)

    # node features pre-scaled by dis (the gather-table rows / self operand)
    xsc = x * dis[:, None]

    # per-core edge lists sharded by dst, sorted by (half, local dst);
    # the gather table is stored in AllGather order: half A = concat of all
    # cores' first HA rows, half B = concat of the rest
    per_core = []
    core_of = dst // NP
    for c in range(NCORES):
        m = core_of == c
        s, dl = src[m], dst[m] - c * NP
        sc = s // NP
        sl = s - sc * NP
        h = (sl >= cfg.HA).astype(np.int64)
        rel = np.where(h == 0, sc * cfg.HA + sl, sc * cfg.HB + (sl - cfg.HA))
        order = np.lexsort((dl, h))
        per_core.append((rel[order], dl[order], h[order]))

    # shared static tile schedule: per (window, half), max tiles over cores
    nwin = cfg.nwin
    cnt = np.zeros((NCORES, nwin, 2), np.int64)
    bounds = []
    for c in range(NCORES):
        s, dl, h = per_core[c]
        nlo = int(np.searchsorted(h, 1))
        blo = np.searchsorted(dl[:nlo], np.arange(nwin + 1) * P)
        bhi = nlo + np.searchsorted(dl[nlo:], np.arange(nwin + 1) * P)
        bounds.append((blo, bhi))
        cnt[c, :, 0] = blo[1:] - blo[:-1]
        cnt[c, :, 1] = bhi[1:] - bhi[:-1]
    tiles_wh = np.max(-(-cnt // P), axis=0)      # [nwin, 2]
    T_lo = int(tiles_wh[:, 0].sum())
    T_hi = int(tiles_wh[:, 1].sum())
    T = T_lo + T_hi

    src_rel = np.zeros((NCORES, P, T), np.int16)
    # dense layer-invariant BINARY selection tiles in fp8 (exact 0/1),
    # streamed from HBM once in layer 0 and SBUF-resident afterwards:
    # selbig[:, t*P:(t+1)*P][r, c] = 1 for tile t's edge in slot r targeting
    # window-local dst c (zeros elsewhere)
    selbig = np.zeros((NCORES, P, T * P), NP8)
    for c in range(NCORES):
        s, dl, h = per_core[c]
        blo, bhi = bounds[c]
        for half in (0, 1):
            t0 = 0 if half == 0 else T_lo
            bb = blo if half == 0 else bhi
            for wi in range(nwin):
                for j in range(int(tiles_wh[wi, half])):
                    a = bb[wi] + j * P
                    n = max(0, min(P, bb[wi + 1] - a))
                    t = t0 + j
                    if n > 0:
                        src_rel[c, :n, t] = s[a:a + n]
                        selbig[c, np.arange(n),
                               t * P + (dl[a:a + n] - wi * P)] = 1.0
                t0 += int(tiles_wh[wi, half])

    # int16 index stream for dma_gather: flat position i -> [i%16, i//16],
    # replicated across the 8 16-partition groups
    idx16 = np.zeros((NCORES, P, 8 * T), np.int16)
    for c in range(NCORES):
        flat = src_rel[c].T.reshape(-1)          # tile-major, then partition
        wrapped = flat.reshape(-1, 16).T         # [16, 8*T]
        idx16[c] = np.tile(wrapped, (8, 1))

    # per-core own-shard pre-scaled features in [node%128, tile*128+d]
    # layout (layer-0 self-loop operand, loaded straight into hnew),
    # zero-padded tail
    NT = nwin
    x_own = np.zeros((NCORES, P, NT * P), np.float16)
    batchf = np.full((NCORES, P, NT), -1.0, np.float32)
    disb = np.zeros((NCORES, P, NT * P), np.float16)
    for c in range(NCORES):
        xs = xsc[c * NP:(c + 1) * NP]
        pad = np.zeros((NT * P - NP, D), np.float32)
        x_own[c] = np.concatenate([xs, pad]).reshape(NT, P, D).transpose(
            1, 0, 2).reshape(P, NT * P).astype(np.float16)
        ids = np.concatenate([batch[c * NP:(c + 1) * NP].astype(np.float32),
                              np.full(NT * P - NP, -1.0, np.float32)])
        batchf[c] = ids.reshape(NT, P).T
        row = np.concatenate([dis[c * NP:(c + 1) * NP],
                              np.zeros(NT * P - NP, np.float32)])
        disb[c] = np.broadcast_to(row.astype(np.float16), (P, NT * P))

    iota = np.broadcast_to(np.arange(512, dtype=np.float16), (P, 512)).copy()
    identf8 = np.eye(P, dtype=np.float32).astype(NP8)
    Wpack = np.asarray(Ws, np.float16).transpose(1, 0, 2).reshape(D, cfg.L * D)
    gb = np.zeros((P, 2 * cfg.L), np.float32)
    for l in range(cfg.L):
        gb[:, 2 * l] = np.asarray(gammas[l], np.float32)
        gb[:, 2 * l + 1] = np.asarray(betas[l], np.float32)
    recip_pk = np.zeros((P, cfg.gblk), np.float32)
    for b in range(cfg.gblk):
        n = min(P, G - b * P)
        recip_pk[:n, b] = recip[b * P:b * P + n]

    xa = np.concatenate([xsc[c * NP:c * NP + cfg.HA]
                         for c in range(NCORES)]).astype(np.float16)
    xb = np.concatenate([xsc[c * NP + cfg.HA:(c + 1) * NP]
                         for c in range(NCORES)]).astype(np.float16)
    shared = dict(xa=np.ascontiguousarray(xa), xb=np.ascontiguousarray(xb),
                  wt=Wpack, gb=gb, iota=iota, recip=recip_pk,
                  identf8=identf8)
    per_core_inputs = [dict(idx16=np.ascontiguousarray(idx16[c]),
                            selbig=np.ascontiguousarray(selbig[c]),
                            batchf=np.ascontiguousarray(batchf[c]),
                            xown=np.ascontiguousarray(x_own[c]),
                            disb=np.ascontiguousarray(disb[c]))
                       for c in range(NCORES)]
    sched = dict(tiles_wh=tiles_wh, T_lo=T_lo, T_hi=T_hi, T=T)
    return shared, per_core_inputs, sched


def build(cfg, sched, debug_dump=False):
    tiles_wh = sched["tiles_wh"]
    T_lo, T_hi, T = sched["T_lo"], sched["T_hi"], sched["T"]
    L, N, G, NP = cfg.L, cfg.N, cfg.G, cfg.NP

    nc = bacc.Bacc("TRN2", target_bir_lowering=False, debug=False,
                   num_devices=NCORES, num_swdge_queues=4,
                   dynamic_dma_scratch_size=16384)
    dbg = {}
    if debug_dump:
        for nm, shape, dt_ in [("zT", [P, cfg.nwin * P], F16),
                               ("hpre", [P, cfg.nblk * WBLK], F16),
                               ("stat", [P, 2], F32),
                               ("hnew", [P, cfg.nwin * P], F16)]:
            for l in range(L):
                dbg[f"{nm}{l}"] = nc.dram_tensor(
                    f"dbg_{nm}{l}", shape, dt_, kind="ExternalOutput")

    xa_e = nc.dram_tensor("xa", [NCORES * cfg.HA, D], F16,
                          kind="ExternalInput")
    xb_e = nc.dram_tensor("xb", [NCORES * cfg.HB, D], F16,
                          kind="ExternalInput")
    wt_e = nc.dram_tensor("wt", [P, L * D], F16, kind="ExternalInput")
    gb_e = nc.dram_tensor("gb", [P, 2 * L], F32, kind="ExternalInput")
    iota_e = nc.dram_tensor("iota", [P, 512], F16, kind="ExternalInput")
    recip_e = nc.dram_tensor("recip", [P, cfg.gblk], F32, kind="ExternalInput")
    identf8_e = nc.dram_tensor("identf8", [P, P], F8, kind="ExternalInput")
    batchf_e = nc.dram_tensor("batchf", [P, cfg.nwin], F32, kind="ExternalInput")
    idx16_e = nc.dram_tensor("idx16", [P, 8 * T], I16, kind="ExternalInput")
    selbig_e = nc.dram_tensor("selbig", [P, T * P], F8, kind="ExternalInput")
    xown_e = nc.dram_tensor("xown", [P, cfg.nwin * P], F16, kind="ExternalInput")
    disb_e = nc.dram_tensor("disb", [P, cfg.nwin * P], F16, kind="ExternalInput")
    out_e = nc.dram_tensor("out", [G, D], F32, kind="ExternalOutput")

    rg = [list(range(NCORES))]

    with tile.TileContext(nc) as tc:
        with tc.tile_pool(name="const", bufs=1) as cp, \
             tc.tile_pool(name="gpool", bufs=6) as gp, \
             tc.tile_pool(name="mselp", bufs=3) as mselp, \
             tc.tile_pool(name="big", bufs=1) as bigp, \
             tc.tile_pool(name="scr", bufs=2) as scrp, \
             tc.tile_pool(name="small", bufs=4) as smp, \
             tc.tile_pool(name="pz", bufs=4, space="PSUM") as pzp, \
             tc.tile_pool(name="ph", bufs=1, space="PSUM") as php, \
             tc.tile_pool(name="pt", bufs=2, space="PSUM") as ptp, \
             tc.tile_pool(name="pg", bufs=1, space="PSUM") as pgp, \
             tc.tile_pool(name="dram", bufs=1, space="DRAM") as dp:

            # ---- constants into SBUF ----
            iota_sb = cp.tile([P, 512], F16)
            nc.sync.dma_start(out=iota_sb[:], in_=iota_e[:, :])
            wt_sb = cp.tile([P, L * D], F16)
            nc.sync.dma_start(out=wt_sb[:], in_=wt_e[:, :])
            gb_sb = cp.tile([P, 2 * L], F32)
            nc.sync.dma_start(out=gb_sb[:], in_=gb_e[:, :])
            recip_sb = cp.tile([P, cfg.gblk], F32)
            nc.sync.dma_start(out=recip_sb[:], in_=recip_e[:, :])
            batchf_sb = cp.tile([P, cfg.nwin], F32)
            nc.sync.dma_start(out=batchf_sb[:], in_=batchf_e[:, :])
            idx_sb = cp.tile([P, 8 * T], I16)
            nc.sync.dma_start(out=idx_sb[:], in_=idx16_e[:, :])
            disb_sb = cp.tile([P, cfg.nwin * P], F16)
            nc.sync.dma_start(out=disb_sb[:], in_=disb_e[:, :])
            identf8_sb = cp.tile([P, P], F8)
            nc.sync.dma_start(out=identf8_sb[:], in_=identf8_e[:, :])
            ident = cp.tile([P, P], F32)
            make_identity(nc, ident[:])
            ident16 = cp.tile([P, P], F16)
            make_identity(nc, ident16[:])
            zero_c = cp.tile([P, 1], F32)
            nc.vector.memset(zero_c[:], 0.0)
            eps_c = cp.tile([P, 1], F32)
            nc.vector.memset(eps_c[:], EPS)

            # layer-invariant binary Sel, SBUF-resident (fp8). Streamed
            # chunk-by-chunk during layer 0, reused by layers 1-2.
            selres = cp.tile([P, T * P], F8)
            sel_loaded = {}

            def ensure_selchunk(gt, l):
                """Stream the prebuilt Sel chunk covering GLOBAL tile gt
                into the resident buffer (layer 0 only)."""
                if l > 0:
                    return
                k = gt // GB_TILES
                if k not in sel_loaded:
                    cnt_t = min(GB_TILES, T - k * GB_TILES)
                    a = k * GB_TILES * P
                    nc.sync.dma_start(
                        out=selres[:, a:a + cnt_t * P],
                        in_=selbig_e[:, a:a + cnt_t * P])
                    sel_loaded[k] = True

            # layer-0 self operand loaded straight into hnew (pre-scaled x)
            hnew = bigp.tile([P, cfg.nwin * P], F16)
            nc.sync.dma_start(out=hnew[:], in_=xown_e[:, :])

            tabA, tabB = [], []
            for l in range(L - 1):
                tabA.append(dp.tile([NCORES * cfg.HA, D], F16,
                                    addr_space="Shared", name=f"tabA{l}"))
                tabB.append(dp.tile([NCORES * cfg.HB, D], F16,
                                    addr_space="Shared", name=f"tabB{l}"))

            # persistent num_idxs registers: one per distinct gather row
            # count, created once so gathers carry no per-instruction MOVE
            # (a fresh MOVE onto the shared register stalls dispatch until
            # the in-flight gather reading it retires)
            nidx_regs = {}
            for T_ph in (T_lo, T_hi):
                k = 0
                while k * GB_TILES < T_ph:
                    v = min(GB_TILES, T_ph - k * GB_TILES) * P
                    if v not in nidx_regs:
                        nidx_regs[v] = nc.gpsimd.to_reg(v)
                    k += 1

            # per-(window,half) phase-local first tile index
            starts = np.zeros((cfg.nwin, 2), np.int64)
            t0 = 0
            for w in range(cfg.nwin):
                starts[w, 0] = t0
                t0 += int(tiles_wh[w][0])
            t0 = 0
            for w in range(cfg.nwin):
                starts[w, 1] = t0
                t0 += int(tiles_wh[w][1])

            for l in range(L):
                srcA = xa_e if l == 0 else tabA[l - 1]
                srcB = xb_e if l == 0 else tabB[l - 1]

                zT = bigp.tile([P, cfg.nwin * P], F16, tag="zT")
                gather_tiles = [{}, {}]
                gq = [0]  # round-robin SWDGE queue over gathers

                def issue_phase_gathers(half, l=l, srcA=srcA, srcB=srcB,
                                        gather_tiles=gather_tiles):
                    """Issue ALL of a phase's dma_gathers back-to-back into
                    fresh pool slots. With no interleaved consumers or slot
                    reuse, the Tile framework attaches each gather's waits to
                    the instruction itself (no standalone EventSemaphore
                    barriers on gpsimd), letting gathers on different SWDGE
                    queues overlap descriptor generation across Q7 core
                    pairs."""
                    T_ph = T_lo if half == 0 else T_hi
                    cache = gather_tiles[half]
                    nk = -(-T_ph // GB_TILES)
                    for k in range(nk):
                        cnt_t = min(GB_TILES, T_ph - k * GB_TILES)
                        g = gp.tile([P, GB_TILES * P], F16, tag="gath")
                        cache[k] = g
                        base = (0 if half == 0 else T_lo) + k * GB_TILES
                        tbl = srcA[:, :] if half == 0 else srcB[:, :]
                        # tiny Act-engine touch: absorbs the slot's
                        # WAR/WAW waits on the near-idle scalar queue, so the
                        # gather carries a single attached wait and the
                        # gpsimd stream is pure gathers (max Q7 in-flight
                        # depth, no standalone EventSemaphore barriers)
                        nc.scalar.activation(out=g[0:1, :1],
                                             in_=zero_c[0:1, :1],
                                             func=ACTF.Copy)
                        nc.gpsimd.dma_gather(
                            out_ap=g[:, :cnt_t * P].rearrange(
                                "p (t d) -> p t d", d=P),
                            in_ap=tbl,
                            idxs_ap=idx_sb[:, base * 8:(base + cnt_t) * 8],
                            num_idxs=cnt_t * P,
                            num_idxs_reg=nidx_regs[cnt_t * P],
                            elem_size=D,
                            single_packet=False,
                            queue_num=gq[0] % 4,
                        )
                        gq[0] += 1

                def ensure_gather(half, t_local, gather_tiles=gather_tiles):
                    return gather_tiles[half][t_local // GB_TILES]

                issue_phase_gathers(0)
                issue_phase_gathers(1)

                # --- lo phase: identity self-loop + lo-half edge tiles ---
                for w in range(cfg.nwin):
                    wlen = cfg.winlens[w]
                    nlo = int(tiles_wh[w][0])
                    pz = pzp.tile([P, P], F32, tag="pz")
                    nc.tensor.matmul(out=pz[:, :wlen],
                                     lhsT=hnew[:, w * P:(w + 1) * P],
                                     rhs=identf8_sb[:, :wlen],
                                     start=True, stop=(nlo == 0))
                    for j in range(nlo):
                        t = int(starts[w, 0]) + j
                        g = ensure_gather(0, t)
                        gt = t                      # global tile index
                        ensure_selchunk(gt, l)
                        slot = t % GB_TILES
                        nc.tensor.matmul(
                            out=pz[:, :wlen],
                            lhsT=g[:, slot * P:(slot + 1) * P],
                            rhs=selres[:, gt * P:gt * P + wlen],
                            start=False, stop=(j == nlo - 1))
                    # evacuate on the near-idle scalar engine: keeps the
                    # PSUM-free chain (which paces gather slot recycling)
                    # off the busier DVE queue
                    nc.scalar.activation(out=zT[:, w * P:w * P + wlen],
                                         in_=pz[:, :wlen], func=ACTF.Copy)

                # --- hi phase: hi-half edge tiles added into zT, with the
                # dis[dst] scale + W matmul + BN-stat evacs of each 512-col
                # block interleaved as soon as its 4 windows complete — this
                # pulls the stats AllReduce (layer-boundary critical path)
                # forward by the whole W/evac tail ---
                hpre = bigp.tile([P, cfg.nblk * WBLK], F16, tag="hpre")
                sums = smp.tile([P, cfg.nblk], F32, tag="sums")
                sqs = smp.tile([P, cfg.nblk], F32, tag="sqs")
                wpw = WBLK // P                      # windows per block
                for w in range(cfg.nwin):
                    wlen = cfg.winlens[w]
                    nhi = int(tiles_wh[w][1])
                    if nhi:
                        pz = pzp.tile([P, P], F32, tag="pz")
                        for j in range(nhi):
                            t = int(starts[w, 1]) + j
                            g = ensure_gather(1, t)
                            gt = T_lo + t           # global tile index
                            ensure_selchunk(gt, l)
                            slot = t % GB_TILES
                            nc.tensor.matmul(
                                out=pz[:, :wlen],
                                lhsT=g[:, slot * P:(slot + 1) * P],
                                rhs=selres[:, gt * P:gt * P + wlen],
                                start=(j == 0), stop=(j == nhi - 1))
                        nc.vector.tensor_tensor(
                            out=zT[:, w * P:w * P + wlen],
                            in0=zT[:, w * P:w * P + wlen],
                            in1=pz[:, :wlen], op=OP.add)
                    for b in range(cfg.nblk):
                        if min((b + 1) * wpw - 1, cfg.nwin - 1) != w:
                            continue
                        blen = cfg.blens[b]
                        sl = slice(b * WBLK, b * WBLK + blen)
                        nc.vector.tensor_tensor(out=zT[:, sl], in0=zT[:, sl],
                                                in1=disb_sb[:, sl],
                                                op=OP.mult)
                        ph = php.tile([P, WBLK], F32, tag="ph")
                        nc.tensor.matmul(
                            out=ph[:, :blen],
                            lhsT=wt_sb[:, l * D:(l + 1) * D],
                            rhs=zT[:, sl],
                            start=True, stop=True)
                        nc.scalar.activation(
                            out=hpre[:, sl],
                            in_=ph[:, :blen], func=ACTF.Copy,
                            accum_out=sums[:, b:b + 1])
                        scr = scrp.tile([P, WBLK], F32, tag="scr")
                        nc.scalar.activation(
                            out=scr[:, :blen], in_=ph[:, :blen],
                            func=ACTF.Square,
                            bias=zero_c[:, :1], accum_out=sqs[:, b:b + 1])

                if debug_dump:
                    nc.sync.dma_start(out=dbg[f"zT{l}"][:, :], in_=zT[:])

                ssum = smp.tile([P, 1], F32, tag="ssum")
                ssq = smp.tile([P, 1], F32, tag="ssq")
                nc.vector.reduce_sum(out=ssum[:], in_=sums[:],
                                     axis=mybir.AxisListType.X)
                nc.vector.reduce_sum(out=ssq[:], in_=sqs[:],
                                     axis=mybir.AxisListType.X)
                statpk = smp.tile([P, 2], F32, tag="statpk")
                nc.vector.tensor_copy(out=statpk[:, 0:1], in_=ssum[:])
                nc.vector.tensor_copy(out=statpk[:, 1:2], in_=ssq[:])
                stat_in = dp.tile([P, 2], F32, name=f"statin{l}")
                stat_out = dp.tile([P, 2], F32, addr_space="Shared",
                                   name=f"statout{l}")
                nc.sync.dma_start(out=stat_in[:], in_=statpk[:])
                nc.gpsimd.collective_compute(
                    "AllReduce", OP.add, replica_groups=rg,
                    ins=[stat_in[:].opt()], outs=[stat_out[:].opt()])
                statred = smp.tile([P, 2], F32, tag="statred")
                nc.sync.dma_start(out=statred[:], in_=stat_out[:])

                if debug_dump:
                    nc.sync.dma_start(out=dbg[f"hpre{l}"][:, :], in_=hpre[:])
                    nc.sync.dma_start(out=dbg[f"stat{l}"][:, :], in_=statred[:])

                mu = smp.tile([P, 1], F32, tag="mu")
                ex2 = smp.tile([P, 1], F32, tag="ex2")
                var = smp.tile([P, 1], F32, tag="var")
                std = smp.tile([P, 1], F32, tag="std")
                rsinv = smp.tile([P, 1], F32, tag="rsinv")
                s1 = smp.tile([P, 1], F32, tag="s1")
                s2 = smp.tile([P, 1], F32, tag="s2")
                inv_n = float(np.float32(1.0 / N))
                nc.vector.tensor_scalar(out=mu[:], in0=statred[:, 0:1],
                                        scalar1=inv_n, scalar2=None,
                                        op0=OP.mult)
                nc.vector.tensor_scalar(out=ex2[:], in0=statred[:, 1:2],
                                        scalar1=inv_n, scalar2=None,
                                        op0=OP.mult)
                nc.vector.scalar_tensor_tensor(
                    out=var[:], in0=mu[:], scalar=1.0, in1=mu[:],
                    op0=OP.bypass, op1=OP.mult)
                nc.vector.tensor_tensor(out=var[:], in0=ex2[:], in1=var[:],
                                        op=OP.subtract)
                nc.scalar.activation(out=std[:], in_=var[:], func=ACTF.Sqrt,
                                     bias=eps_c[:, :1])
                nc.vector.reciprocal(out=rsinv[:], in_=std[:])
                nc.vector.tensor_tensor(out=s1[:], in0=gb_sb[:, 2 * l:2 * l + 1],
                                        in1=rsinv[:], op=OP.mult)
                nc.vector.tensor_tensor(out=s2[:], in0=mu[:], in1=s1[:],
                                        op=OP.mult)
                nc.vector.tensor_tensor(out=s2[:],
                                        in0=gb_sb[:, 2 * l + 1:2 * l + 2],
                                        in1=s2[:], op=OP.subtract)

                # ---- normalize (+relu; + dis pre-scale except last layer),
                # transpose back to [node, D] ----
                hnorm = bigp.tile([P, cfg.nwin * P], F16, tag="zT")
                for b in range(cfg.nblk):
                    blen = cfg.blens[b]
                    sl = slice(b * WBLK, b * WBLK + blen)
                    if l < L - 1:
                        nc.scalar.activation(out=hnorm[:, sl], in_=hpre[:, sl],
                                             func=ACTF.Relu, bias=s2[:, :1],
                                             scale=s1[:, :1])
                        nc.vector.tensor_tensor(out=hnorm[:, sl],
                                                in0=hnorm[:, sl],
                                                in1=disb_sb[:, sl],
                                                op=OP.mult)
                    else:
                        nc.vector.tensor_scalar(out=hnorm[:, sl],
                                                in0=hpre[:, sl],
                                                scalar1=s1[:, :1],
                                                scalar2=s2[:, :1],
                                                op0=OP.mult, op1=OP.add)
                if l == L - 1:
                    pgps = pgp.tile([P, G], F32)
                for nt in range(cfg.nwin):
                    tl = cfg.winlens[nt]
                    pt = ptp.tile([P, P], F16, tag="pt")
                    nc.tensor.transpose(out=pt[:tl, :],
                                        in_=hnorm[:, nt * P:nt * P + tl],
                                        identity=ident16[:])
                    nc.vector.tensor_copy(out=hnew[:tl, nt * P:(nt + 1) * P],
                                          in_=pt[:tl, :])
                    if l == L - 1:
                        # global-mean-pool matmul for this window right after
                        # its transpose lands — the pool PSUM completes with
                        # the last transpose instead of serializing after
                        msel = mselp.tile([P, G], F16, tag="msel")
                        nc.vector.tensor_scalar(
                            out=msel[:], in0=iota_sb[:, :G],
                            scalar1=batchf_sb[:, nt:nt + 1],
                            scalar2=None, op0=OP.is_equal)
                        nc.tensor.matmul(out=pgps[:],
                                         lhsT=hnew[:, nt * P:(nt + 1) * P],
                                         rhs=msel[:], start=(nt == 0),
                                         stop=(nt == cfg.nwin - 1))
                    if l < L - 1 and nt == cfg.NFA - 1:
                        # half A written: AllGather it now so the next
                        # layer's phase-A gathers overlap with AG of half B
                        partA = dp.tile([cfg.HA, D], F16, name=f"partA{l}")
                        nc.sync.dma_start(
                            out=partA[:, :].rearrange("(nt p) d -> p nt d",
                                                      p=P),
                            in_=hnew[:, :cfg.NFA * P].rearrange(
                                "p (nt d) -> p nt d", d=D))
                        nc.gpsimd.collective_compute(
                            "AllGather", OP.bypass, replica_groups=rg,
                            ins=[partA[:].opt()], outs=[tabA[l][:].opt()])
                if debug_dump:
                    nc.sync.dma_start(out=dbg[f"hnew{l}"][:, :], in_=hnew[:])

                if l < L - 1:
                    partB = dp.tile([cfg.HB, D], F16, name=f"partB{l}")
                    nfb = cfg.nfull - cfg.NFA
                    if nfb:
                        nc.sync.dma_start(
                            out=partB[:nfb * P, :].rearrange(
                                "(nt p) d -> p nt d", p=P),
                            in_=hnew[:, cfg.NFA * P:cfg.nfull * P].rearrange(
                                "p (nt d) -> p nt d", d=D))
                    if cfg.rem:
                        nc.sync.dma_start(
                            out=partB[nfb * P:, :],
                            in_=hnew[:cfg.rem,
                                     cfg.nfull * P:(cfg.nfull + 1) * P])
                    nc.gpsimd.collective_compute(
                        "AllGather", OP.bypass, replica_groups=rg,
                        ins=[partB[:].opt()], outs=[tabB[l][:].opt()])

            # ---- global mean pool (matmuls were interleaved with the last
            # layer's transposes above) ----
            poolsb = cp.tile([P, G], F32)
            nc.vector.tensor_copy(out=poolsb[:], in_=pgps[:])
            pool_in = dp.tile([P, G], F32, name="poolin")
            pool_out = dp.tile([P, G], F32, addr_space="Shared", name="poolout")
            nc.sync.dma_start(out=pool_in[:], in_=poolsb[:])
            nc.gpsimd.collective_compute(
                "AllReduce", OP.add, replica_groups=rg,
                ins=[pool_in[:].opt()], outs=[pool_out[:].opt()])
            poolred = cp.tile([P, G], F32)
            nc.sync.dma_start(out=poolred[:], in_=pool_out[:])
            outsb = cp.tile([P, cfg.gblk * D], F32)
            for b in range(cfg.gblk):
                gl = min(P, G - b * P)
                pt = ptp.tile([P, P], F32, tag="pt")
                nc.tensor.transpose(out=pt[:gl, :],
                                    in_=poolred[:, b * P:b * P + gl],
                                    identity=ident[:])
                nc.vector.tensor_scalar(out=outsb[:gl, b * D:(b + 1) * D],
                                        in0=pt[:gl, :],
                                        scalar1=recip_sb[:gl, b:b + 1],
                                        scalar2=None, op0=OP.mult)
            if cfg.gblk == 1:
                nc.sync.dma_start(out=out_e[:, :], in_=outsb[:G, :D])
            else:
                nc.sync.dma_start(
                    out=out_e[:, :].rearrange("(b g) d -> g b d", g=P),
                    in_=outsb[:, :].rearrange("g (b d) -> g b d", d=D))
    nc.compile()
    return nc


_CACHE = {}


def _get_compiled(cfg, sched_key, sched, debug_dump=False):
    key = (cfg.N, cfg.E, cfg.G, cfg.L, sched_key, debug_dump)
    if key not in _CACHE:
        _CACHE[key] = build(cfg, sched, debug_dump=debug_dump)
    return _CACHE[key]


def run(cfg, inputs, trace=False, debug_dump=False):
    shared, per_core, sched = host_preprocess(cfg, **inputs)
    sched_key = (sched["T_lo"], sched["T_hi"],
                 tuple(map(tuple, sched["tiles_wh"])))
    nc = _get_compiled(cfg, sched_key, sched, debug_dump=debug_dump)
    in_maps = [dict(shared, **pc) for pc in per_core]
    res = bass_utils.run_bass_kernel_spmd(
        nc, in_maps, core_ids=list(range(NCORES)), trace=trace)
    out = res.results[0]["out"]
    return out, res


def build_null(cfg, sched):
    """Same external I/O signature as build(), trivial compute — used to
    subtract host/RPC/dispatch overhead from wall-clock timing."""
    T = sched["T"]
    L, N, G = cfg.L, cfg.N, cfg.G
    nc = bacc.Bacc("TRN2", target_bir_lowering=False, debug=False,
                   num_devices=NCORES)
    nc.dram_tensor("xa", [NCORES * cfg.HA, D], F16, kind="ExternalInput")
    nc.dram_tensor("xb", [NCORES * cfg.HB, D], F16, kind="ExternalInput")
    nc.dram_tensor("wt", [P, L * D], F16, kind="ExternalInput")
    nc.dram_tensor("gb", [P, 2 * L], F32, kind="ExternalInput")
    iota_e = nc.dram_tensor("iota", [P, 512], F16, kind="ExternalInput")
    nc.dram_tensor("recip", [P, cfg.gblk], F32, kind="ExternalInput")
    nc.dram_tensor("identf8", [P, P], F8, kind="ExternalInput")
    nc.dram_tensor("batchf", [P, cfg.nwin], F32, kind="ExternalInput")
    nc.dram_tensor("idx16", [P, 8 * T], I16, kind="ExternalInput")
    nc.dram_tensor("selbig", [P, T * P], F8, kind="ExternalInput")
    nc.dram_tensor("xown", [P, cfg.nwin * P], F16, kind="ExternalInput")
    nc.dram_tensor("disb", [P, cfg.nwin * P], F16, kind="ExternalInput")
    out_e = nc.dram_tensor("out", [G, D], F32, kind="ExternalOutput")
    with tile.TileContext(nc) as tc:
        with tc.tile_pool(name="sb", bufs=1) as sb:
            t = sb.tile([P, D], F16)
            nc.sync.dma_start(out=t[:], in_=iota_e[:, :D])
            t32 = sb.tile([P, D], F32)
            nc.vector.tensor_copy(out=t32[:], in_=t[:])
            for b in range(-(-G // P)):
                gl = min(P, G - b * P)
                nc.sync.dma_start(out=out_e[b * P:b * P + gl, :],
                                  in_=t32[:gl, :])
    nc.compile()
    return nc


def _make_pjrt_fn(nc, in_maps, chain=1, raw=False):
    """Compile the program via PJRT and return a zero-arg callable that
    executes it `chain` times back-to-back (each call's outputs feed the
    next call's output operands, forcing serial device execution) with
    device-resident inputs."""
    import jax
    from jax.sharding import Mesh, PartitionSpec, NamedSharding
    from jax.experimental.shard_map import shard_map
    from concourse import bass2jax
    from concourse import mybir as mb

    bass2jax.install_neuronx_cc_hook()
    partition_name = (nc.partition_id_tensor.name
                      if nc.partition_id_tensor else None)
    in_names, out_names, out_avals, zero_outs = [], [], [], []
    for alloc in nc.m.functions[0].allocations:
        if not isinstance(alloc, mb.MemoryLocationSet):
            continue
        name = alloc.memorylocations[0].name
        if alloc.kind == "ExternalInput":
            if name != partition_name:
                in_names.append(name)
        elif alloc.kind == "ExternalOutput":
            out_names.append(name)
            shape = tuple(alloc.tensor_shape)
            dtype = mb.dt.np(alloc.dtype)
            out_avals.append(jax.core.ShapedArray(shape, dtype))
            zero_outs.append(np.zeros(shape, dtype))
    n_params = len(in_names)
    in_names = in_names + out_names
    if partition_name is not None:
        in_names.append(partition_name)

    def _body(*args):
        params = list(args[:n_params])
        outs = list(args[n_params:])
        pid = ([bass2jax.partition_id_tensor()]
               if partition_name is not None else [])
        for _ in range(chain):
            outs = list(bass2jax._bass_exec_p.bind(
                *params, *outs, *pid,
                out_avals=tuple(out_avals), in_names=tuple(in_names),
                out_names=tuple(out_names), lowering_input_output_aliases=(),
                sim_require_finite=True, sim_require_nnan=True, nc=nc))
        return tuple(outs)

    devices = jax.devices()[:NCORES]
    mesh = Mesh(np.asarray(devices), ("core",))
    spec = PartitionSpec("core")
    in_specs = (spec,) * (n_params + len(out_names))
    out_specs = (spec,) * len(out_names)
    fn = jax.jit(shard_map(_body, mesh=mesh, in_specs=in_specs,
                           out_specs=out_specs, check_rep=False),
                 keep_unused=True)
    sharding = NamedSharding(mesh, spec)
    concat_in = [
        jax.device_put(np.concatenate(
            [np.asarray(in_maps[c][in_names[i]]) for c in range(NCORES)],
            axis=0), sharding)
        for i in range(n_params)
    ]
    concat_zeros = [
        jax.device_put(np.zeros((NCORES * z.shape[0], *z.shape[1:]), z.dtype),
                       sharding)
        for z in zero_outs
    ]
    jax.block_until_ready(concat_in)
    if raw:
        return fn, concat_in, concat_zeros

    def call():
        jax.block_until_ready(fn(*concat_in, *concat_zeros))

    return call


def time_pjrt(nc, in_maps, iters=8, warmup=2, chain=1):
    """Wall-clock repeated executions of the compiled program with
    device-resident inputs (mirrors bass2jax.run_bass_via_pjrt)."""
    import time
    call = _make_pjrt_fn(nc, in_maps, chain=chain)
    for _ in range(warmup):
        call()
    times = []
    for _ in range(iters):
        t0 = time.perf_counter()
        call()
        times.append(time.perf_counter() - t0)
    return times


def measure(cfg, inputs, iters=24):
    """Interleaved blocked kernel/null executions. The ~82ms axon RPC round
    trip dominates each call, so the kernel time is estimated as the trimmed
    mean of paired (kernel - null) wall differences; interleaving cancels
    slow drift. Returns (kernel_walls, null_walls) as paired lists."""
    import time
    import jax
    shared, per_core, sched = host_preprocess(cfg, **inputs)
    sched_key = (sched["T_lo"], sched["T_hi"],
                 tuple(map(tuple, sched["tiles_wh"])))
    nc = _get_compiled(cfg, sched_key, sched)
    in_maps = [dict(shared, **pc) for pc in per_core]
    key = ("null", cfg.N, cfg.E, cfg.G, sched_key)
    if key not in _CACHE:
        _CACHE[key] = build_null(cfg, sched)
    fnk, ink, zk = _make_pjrt_fn(nc, in_maps, raw=True)
    fnn, inn, zn = _make_pjrt_fn(_CACHE[key], in_maps, raw=True)
    jax.block_until_ready(fnk(*ink, *zk))
    jax.block_until_ready(fnn(*inn, *zn))
    ks, ns = [], []
    for _ in range(iters):
        t0 = time.perf_counter()
        jax.block_until_ready(fnk(*ink, *zk))
        ks.append(time.perf_counter() - t0)
        t0 = time.perf_counter()
        jax.block_until_ready(fnn(*inn, *zn))
        ns.append(time.perf_counter() - t0)
    return ks, ns


def kernel(x, edge_index, batch, Ws, bs, gammas, betas):
    cfg = Cfg(N=50000, E=625000, G=256, L=3)
    out, _ = run(cfg, dict(x=x, edge_index=edge_index, batch=batch, Ws=Ws,
                           bs=bs, gammas=gammas, betas=betas))
    return out
